# revision 11
# baseline (speedup 1.0000x reference)
"""Trainium2 Bass kernel for the BottleNeck-involution block — 128-partition
repack.

Sharding: pure data parallel over (batch=4) x (H halves) = 8 shards, one per
NeuronCore.  Each core computes a (1, 128, 48, 96) slice of the output.

Layout: partitions hold (rb, js): rb = 4 row-blocks of 12 rows, js = 32 cols
of a strip; 3 strips s cover the 96 cols.  All 128 partitions carry involution
work (vs 96 in the j-only packing), so every engine's per-element cost drops
by 4/3.  Free dim rasters:
  xs[kj]  [128, (ii 18, s 3, c' 64)]   x*s_i, 7 col-shifted copies (bf16)
  kern_s  [128, (i 12, s 3, kk*4+g)]   per-pixel involution kernels (bf16)
  acc     [128, (i 4, s 3, c' 64)]     PSUM accumulator, one of 3 row-segs
  t_sb    [17, (i 48, j 96)]           reduced features + ones row
  rhs     [128=(x1_cm 64 | x_cm 64), (i, j)]  tail 1x1-conv operand

Per-core pipeline:
  t       = relu(bn_r(w_reduce @ x))     PE + ACT  (channel-major, 6 chunks)
  kern    = [t;1]^T @ [wspan; bspan]     PE        (36 units, (rb,js)-packed)
  3 segs x 49 taps:
    pr    = kern(bcast) * xs[kj]         DVE (38) / Pool (11)  bf16 2x mode
    acc   = b_i + sum pr                 PE identity-matmul PSUM accum
    x1    = gelu(acc)                    ACT
  tail (per seg, interleaved into next seg's taps):
    x1_cm = transpose(x1)                PE + ACT copies
    out   = gelu(wce @ [x1;x] + btail)   PE + ACT, DMA out
"""

import os
import sys

sys.path.insert(0, "/opt/trn_rl_repo")
KDEBUG = int(os.environ.get("KDEBUG", "0"))

import numpy as np

import concourse.bass as bass
from concourse import bacc
import concourse.mybir as mybir
import concourse.tile as tile
from concourse.bass_utils import run_bass_kernel_spmd

F32 = mybir.dt.float32
BF16 = mybir.dt.bfloat16

EPS = 1e-5
KS = 7            # involution kernel size
KK = KS * KS      # 49 taps
GC = 16           # channels per involution group
G = 4             # groups
CR = 16           # reduced channels
B, C, H, W = 4, 64, 96, 96
CO = 128
NCORES = 8

ROWS = H // 2     # 48 output rows per core
PAD = 3
NRB = 4           # row-blocks per core
RBR = ROWS // NRB  # 12 rows per block
NS = 3            # col strips
SW = W // NS      # 32 cols per strip... (strip width must divide 128/NRB)
IROWS = RBR + 2 * PAD  # 18 input rows per block (with halo)
NSEG = 3
SEGI = RBR // NSEG     # 4 i_rel rows per segment
SEGW = SEGI * NS * C   # 768 acc cols per segment
PXW = ROWS * W         # 4608 pixels per core

# taps in kj-major order so products start as each shifted x copy lands
TAPS = [(ki, kj) for kj in range(KS) for ki in range(KS)]
NPOOL = 11
POOL_SLOTS = frozenset(round(3 + m * 45 / (NPOOL - 1)) for m in range(NPOOL))
# last segment runs one fewer Pool tap so Pool drains before DVE
POOL_SLOTS_LAST = frozenset(round(3 + m * 45 / 9) for m in range(10))
POOL_BY_SEG = (POOL_SLOTS, POOL_SLOTS, POOL_SLOTS_LAST)

# blob f32-slot layout: name -> (width, partitions)
_REGIONS = [
    ("xcm", RBR * W, 128),            # x channel-major bf16 [128, 24*96]
    ("c128", 108, 128),               # wredt bf16 | screl f32 | wspan bf16
    ("tones", PXW // 2, 1),           # ones bf16 [1, 4608] (t_sb row 16)
    ("c1b", 448, 1),                  # birow bf16 | ones1 bf16
    ("iden", CO // 2, 128),           # identity bf16 [128, 128]
] + [
    (f"xs{kj}", IROWS * NS * C // 2, 128) for kj in range(KS)
] + [
    ("xcmt", PXW // 2, 128),          # x channel-major bf16 at parts 64-127
    ("wce", CO // 2, 128),            # [wconv; wmap] stacked bf16 [128, 128]
    ("btail", 1, 128),                # tail gelu bias f32 [128, 1]
]
_OFF = {}
_o = 0
for _name, _w, _p in _REGIONS:
    _OFF[_name] = (_o, _o + _w)
    _o += _w
BLOBW = _o

_CACHE = {}


def _build_bass():
    nc = bacc.Bacc()

    blob_d = nc.dram_tensor("blob", [128, BLOBW], F32, kind="ExternalInput")
    out_d = nc.dram_tensor("out", [CO, PXW], BF16, kind="ExternalOutput")
    if KDEBUG:
        tdbg_d = nc.dram_tensor("tdbg", [17, PXW // 2], F32, kind="ExternalOutput")
        kdbg_d = nc.dram_tensor(
            "kdbg", [128, RBR * NS * KK * G // 2], F32, kind="ExternalOutput")
        xdbg_d = nc.dram_tensor(
            "xdbg", [128, RBR * NS * C // 2], F32, kind="ExternalOutput")

    def breg(name, p0=0, p1=None):
        a, b_ = _OFF[name]
        np_ = dict((n, pp) for n, _, pp in _REGIONS)[name] if p1 is None else p1
        return blob_d[p0:np_, a:b_]

    with tile.TileContext(nc) as tc:
        with (
            tc.tile_pool(name="work", bufs=1) as work,
            tc.tile_pool(name="prodD", bufs=10) as prodD,
            tc.tile_pool(name="prodP", bufs=6) as prodP,
            tc.tile_pool(name="outp", bufs=3) as outp,
        ):
            # ---- SBUF tiles + loads (DMA queue order = priority order) ----
            iden = work.tile([128, CO], BF16, name="iden")
            nc.sync.dma_start(iden[:].bitcast(F32), breg("iden"))
            x_cm = work.tile([128, RBR * 2 * W], BF16, name="x_cm")
            nc.sync.dma_start(x_cm[:].bitcast(F32), breg("xcm"))
            c128 = work.tile([128, 108], F32, name="c128")
            nc.sync.dma_start(c128[:], breg("c128"))
            wredt = c128[:, 0:8].bitcast(BF16)
            screl = c128[0:16, 8:10]
            wspan = c128[0:17, 10:108].bitcast(BF16)
            t_sb = work.tile([17, PXW], BF16, name="t_sb")
            nc.sync.dma_start(t_sb[16:17, :].bitcast(F32), breg("tones"))

            xs = []
            for kj in range(KS):
                t_ = work.tile([128, IROWS * NS * C], BF16, name=f"xs{kj}")
                xs.append(t_)

            nc.sync.dma_start(xs[0][:].bitcast(F32), breg("xs0"))
            c1b = work.tile([1, 448], F32, name="c1b")
            nc.sync.dma_start(c1b[:], breg("c1b"))
            birow = c1b[:, 0:384].bitcast(BF16)
            ones1 = c1b[:, 384:448].bitcast(BF16)
            for kj in range(1, KS):
                nc.sync.dma_start(xs[kj][:].bitcast(F32), breg(f"xs{kj}"))

            rhs = work.tile([128, PXW], BF16, name="rhs")
            a0, a1 = _OFF["xcmt"]
            nc.sync.dma_start(
                rhs[64:128, :].bitcast(F32), blob_d[64:128, a0:a1])
            wce = work.tile([128, CO], BF16, name="wce")
            nc.sync.dma_start(wce[:].bitcast(F32), breg("wce"))
            btail = work.tile([128, 1], F32, name="btail")
            nc.sync.dma_start(btail[:], breg("btail"))

            kern_s = work.tile([128, RBR * NS * KK * G], BF16, name="kern_s")
            x1_pm = work.tile([128, RBR * NS * C], BF16, name="x1_pm")
            scr = work.tile([16, 1], F32, name="scr")

            # ---- PSUM pools (LIFO): acc > kps > tps ----
            acc_cm = tc.tile_pool(name="accp", bufs=2, space="PSUM")
            accp = acc_cm.__enter__()
            kps_cm = tc.tile_pool(name="kpsp", bufs=2, space="PSUM")
            kpsp = kps_cm.__enter__()
            tps_cm = tc.tile_pool(name="tpsp", bufs=2, space="PSUM")
            tpsp = tps_cm.__enter__()

            # dummy gelu: force the gelu_and_others act table load at t=0
            nc.scalar.activation(
                scr[:, 0:1], screl[:, 0:1],
                mybir.ActivationFunctionType.Gelu,
            )

            # PE warmup: ramp the tensor engine to full p-state during the
            # x_cm DMA wait (iden lands first; ~20 x 128-cycle matmuls)
            wrm = tpsp.tile([16, 384], F32, name="tps", tag="tps")
            for _ in range(17):
                nc.tensor.matmul(
                    wrm[0:16, 0:128], iden[:, 0:16], iden[:],
                    start=True, stop=True,
                )

            # t_sb free raster: (i_rel 12, s 3, rb 4, js 32) — kern-unit slices
            # are contiguous 128-pixel runs (matmul APs allow 1 free dim)
            t_pk = t_sb[:].rearrange("p (i s r j) -> p r i s j",
                                     i=RBR, s=NS, r=NRB)

            def t_chunk(q, dve_relu=False):
                """t rows {12b' + 2q + e} of both halves (x_cm is host-ordered
                (q, b', e, j) per half so the rhs slice is contiguous).
                bn_r scale is folded into wredt on the host; relu applies the
                bias, on ACT or (dve_relu, for halves h=1) DVE."""
                for h in range(2):
                    tp = tpsp.tile([16, 384], F32, name="tps", tag="tps")
                    nc.tensor.matmul(
                        tp[:],
                        wredt[64 * h:64 * h + 64, :],
                        x_cm[64 * h:64 * h + 64, 384 * q:384 * (q + 1)],
                        start=True, stop=True,
                    )
                    dst = t_pk[0:16, 2 * h:2 * h + 2, 2 * q:2 * q + 2, :, :]
                    if dve_relu and h == 1:
                        nc.vector.tensor_scalar(
                            out=dst, in0=tp[:],
                            scalar1=screl[:, 1:2], scalar2=0.0,
                            op0=mybir.AluOpType.add,
                            op1=mybir.AluOpType.max,
                        )
                    else:
                        nc.scalar.activation(
                            dst, tp[:],
                            mybir.ActivationFunctionType.Relu,
                            bias=screl[:, 1:2],
                            scale=1.0,
                        )

            def kern_pair(p, dve_copy=False):
                """produce kern units 2p, 2p+1 (196 cols each)."""
                kp = kpsp.tile([128, 392], F32, name="kps", tag="kps")
                for s_ in range(2):
                    u = 2 * p + s_
                    nc.tensor.matmul(
                        kp[:, s_ * KK * G:(s_ + 1) * KK * G],
                        t_sb[0:17, 128 * u:128 * (u + 1)],
                        wspan,
                        start=True, stop=True,
                    )
                dst = kern_s[:, 2 * p * KK * G:(2 * p + 2) * KK * G]
                if dve_copy:
                    nc.vector.tensor_copy(dst, kp[:])
                else:
                    nc.scalar.activation(
                        dst, kp[:], mybir.ActivationFunctionType.Identity)

            # ---- lead-in: t chunks 0-1, kern units 0-11 ----
            t_chunk(0)
            kern_pair(0, dve_copy=True)
            kern_pair(1, dve_copy=True)
            t_chunk(1, dve_relu=True)
            kern_pair(2, dve_copy=True)

            # tail pools opened lazily after kps/tps close
            tail_pools = {}

            def tail_a(seg, uu, dve_copy, eng=None):
                """transpose + copy for unit uu of segment seg."""
                tp2p = tail_pools["tp2p"]
                ig = 4 * seg + uu // NS
                st = uu % NS
                ucol = (ig * NS + st) * C
                tp2 = tp2p.tile([64, 128], BF16, name="tp2", tag="tp2")
                nc.tensor.transpose(
                    tp2[:], x1_pm[:, ucol:ucol + C], iden[:])
                dst = rhs[0:64, :].rearrange(
                    "p (b r j) -> p b r j", b=NRB, r=RBR
                )[:, :, ig, SW * st:SW * st + SW]
                if eng is not None:
                    eng.tensor_copy(dst, tp2[:])
                elif dve_copy:
                    nc.vector.tensor_copy(dst, tp2[:])
                else:
                    nc.scalar.activation(
                        dst, tp2[:], mybir.ActivationFunctionType.Identity)

            def tail_b(seg, rb, dma_eng=None):
                """1x1 convs + gelu + out DMA for rows 12rb+4seg..+4."""
                tlp = tail_pools["tlp"]
                px0 = (RBR * rb + SEGI * seg) * W
                pxw = SEGI * W
                tl = tlp.tile([128, pxw], F32, name="tailps", tag="tailps")
                nc.tensor.matmul(
                    tl[:], wce[:], rhs[:, px0:px0 + pxw],
                    start=True, stop=True,
                )
                ost = outp.tile([128, pxw], BF16, name="ost", tag="ost")
                nc.scalar.activation(
                    ost[:], tl[:],
                    mybir.ActivationFunctionType.Gelu,
                    bias=btail[:],
                    scale=1.0,
                )
                (dma_eng or nc.sync).dma_start(out_d[:, px0:px0 + pxw], ost[:])

            def product(seg, slot, pr, i0, i1):
                """tap product for i_loc range [i0, i1) into pr's slice."""
                ki, kj = TAPS[slot]
                kk = ki * KS + kj
                n = i1 - i0
                pr4 = pr[:].rearrange(
                    "p (i s u g) -> p i s u g", i=SEGI, s=NS, u=GC
                )[:, i0:i1]
                b0 = SEGI * seg + ki + i0
                in0 = xs[kj][:].rearrange(
                    "p (i s c) -> p i s c", i=IROWS, s=NS
                )[:, b0:b0 + n, :, :].rearrange("p i s (u g) -> p i s u g", g=G)
                in1 = kern_s[:].rearrange(
                    "p (i s k) -> p i s k", i=RBR, s=NS
                )[:, SEGI * seg + i0:SEGI * seg + i1, :, kk * G:(kk + 1) * G]
                in1b = in1.unsqueeze(3).to_broadcast([128, n, NS, GC, G])
                eng = nc.gpsimd if slot in POOL_BY_SEG[seg] else nc.vector
                eng.tensor_tensor(out=pr4, in0=in0, in1=in1b,
                                  op=mybir.AluOpType.mult)

            # ---- segments ----
            for seg in range(NSEG):
                acc = accp.tile([128, SEGW], F32, name="acc", tag="acc")
                for c0, c1 in ((0, 512), (512, SEGW)):
                    nc.tensor.matmul(
                        acc[:, c0:c1], ones1, birow[:, c0:c1],
                        start=True, stop=False,
                    )
                pend = []
                lag = [0]

                def flush(limit, acc=acc, pend=pend):
                    while len(pend) > limit:
                        pr_, last_ = pend.pop(0)
                        for c0, c1 in ((0, 512), (512, SEGW)):
                            nc.tensor.matmul(
                                acc[:, c0:c1], iden[:], pr_[:, c0:c1],
                                start=False, stop=last_,
                            )

                first = 0
                if seg == 0:
                    # startup: all kj0 taps as half-products — half A needs
                    # only kern pairs 0-2, so the vec engines start as soon
                    # as xs0 lands (seg0 is xs-DMA-paced; the extra per-op
                    # init hides in the DMA wait). Pairs 3-5 copy on ACT in
                    # parallel, then half B.
                    first = KS
                    prs = []
                    for slot in range(KS):
                        pp = prodP if slot in POOL_SLOTS else prodD
                        pr = pp.tile([128, SEGW], BF16, name="prod",
                                     tag="prod")
                        prs.append(pr)
                        product(0, slot, pr, 0, SEGI // 2)
                    for p in range(3, 6):
                        kern_pair(p, dve_copy=False)
                    t_chunk(2)
                    t_chunk(3)
                    for slot in range(KS):
                        product(0, slot, prs[slot], SEGI // 2, SEGI)
                        pend.append((prs[slot], slot == 98))
                        if slot in POOL_SLOTS:
                            lag[0] = 2
                        flush(lag[0])
                        if lag[0] > 0 and slot not in POOL_SLOTS:
                            lag[0] -= 1

                for slot in range(first, KK):
                    pool_tap = slot in POOL_BY_SEG[seg]
                    pp = prodP if pool_tap else prodD
                    pr = pp.tile([128, SEGW], BF16, name="prod", tag="prod")
                    product(seg, slot, pr, 0, SEGI)
                    pend.append((pr, slot == KK - 1))
                    if pool_tap:
                        lag[0] = 2
                    flush(lag[0])
                    if lag[0] > 0 and not pool_tap:
                        lag[0] -= 1

                    # interleaved extras
                    if seg == 0:
                        if 7 <= slot < 9:
                            t_chunk(slot - 3)
                        if slot >= 7 and (slot - 7) % 3 == 0 \
                                and (slot - 7) // 3 < 12:
                            kern_pair(6 + (slot - 7) // 3, dve_copy=False)
                    elif "tp2p" in tail_pools:
                        ps = seg - 1
                        if slot % 3 == 1 and slot // 3 < 12:
                            tail_a(ps, slot // 3, dve_copy=False)
                        if slot >= 37 and (slot - 37) % 3 == 0 \
                                and (slot - 37) // 3 < 4:
                            tail_b(ps, (slot - 37) // 3)
                flush(0)
                for g0, g1 in ((0, 192), (192, 384), (384, SEGW)):
                    nc.scalar.activation(
                        x1_pm[:, SEGW * seg + g0:SEGW * seg + g1],
                        acc[:, g0:g1],
                        mybir.ActivationFunctionType.Gelu,
                    )
                if seg == 0:
                    # kern + t complete: close tps/kps, open tail pools
                    tps_cm.__exit__(None, None, None)
                    kps_cm.__exit__(None, None, None)
                    tl_cm = tc.tile_pool(name="tlp", bufs=1, space="PSUM")
                    tail_pools["tlp"] = tl_cm.__enter__()
                    tail_pools["tl_cm"] = tl_cm
                    tp2_cm = tc.tile_pool(name="tp2p", bufs=3, space="PSUM")
                    tail_pools["tp2p"] = tp2_cm.__enter__()
                    tail_pools["tp2_cm"] = tp2_cm

            # trailing tail (segment 2): every out chunk needs all 12
            # units; copies split 3-way (DVE/ACT/Pool all idle by now).
            for uu in range(RBR):
                tail_a(NSEG - 1, uu, dve_copy=(uu % 4 != 0))
            # out chunks in 2 groups of 2; each group's matmuls land in one
            # 2-bank acc-pool tile (dsts at 0 and 512 to stay bank-aligned),
            # then a single strided-source gelu and a single batched DMA.
            ostf = outp.tile([128, NRB * SEGI * W], BF16, name="ostf",
                             tag="ostf")
            pxw = SEGI * W
            for grp in range(2):
                tl = accp.tile([128, 1024], F32, name="acc", tag="acc")
                for k in range(2):
                    rb = 2 * grp + k
                    px0 = (RBR * rb + SEGI * (NSEG - 1)) * W
                    nc.tensor.matmul(
                        tl[:, 512 * k:512 * k + pxw], wce[:],
                        rhs[:, px0:px0 + pxw],
                        start=True, stop=True,
                    )
                gsrc = tl[:].rearrange("p (k x) -> p k x", k=2)[:, :, 0:pxw]
                nc.scalar.activation(
                    ostf[:, grp * 2 * pxw:(grp + 1) * 2 * pxw], gsrc,
                    mybir.ActivationFunctionType.Gelu,
                    bias=btail[:],
                    scale=1.0,
                )
                odst = out_d[:].rearrange(
                    "p (rb x) -> p rb x", rb=NRB
                )[:, 2 * grp:2 * grp + 2, SEGI * (NSEG - 1) * W:]
                nc.sync.dma_start(
                    odst, ostf[:, grp * 2 * pxw:(grp + 1) * 2 * pxw])

            if KDEBUG:
                nc.sync.dma_start(tdbg_d[:, :], t_sb[:].bitcast(F32))
                nc.sync.dma_start(kdbg_d[:, :], kern_s[:].bitcast(F32))
                nc.sync.dma_start(xdbg_d[:, :], x1_pm[:].bitcast(F32))

            tail_pools["tp2_cm"].__exit__(None, None, None)
            tail_pools["tl_cm"].__exit__(None, None, None)
            acc_cm.__exit__(None, None, None)

    if not nc.is_finalized():
        nc.finalize()
    return nc


def _bf16_pack(arr):
    """bf16-cast a [P, N] array and pack into [P, N/2] f32 slots."""
    import ml_dtypes

    a = np.ascontiguousarray(np.asarray(arr, np.float32)).astype(ml_dtypes.bfloat16)
    return a.view(np.float32)


def _prep_blob_consts(w_reduce, g_r, b_r, m_r, v_r, w_span, b_span,
                      g_i, b_i, m_i, v_i, w_conv, g_c, b_c, m_c, v_c,
                      w_map, b_map, g_m, b_m, m_m, v_m, perm):
    f = np.float32

    def bn_fold(g, b, m, v):
        s = g / np.sqrt(v + EPS)
        return s.astype(f), (b - m * s).astype(f)

    sc_r, bi_r = bn_fold(g_r, b_r, m_r, v_r)
    sc_i, bi_i = bn_fold(g_i, b_i, m_i, v_i)
    sc_c, bi_c = bn_fold(g_c, b_c, m_c, v_c)
    sc_m, bi_m = bn_fold(g_m, b_m, m_m, v_m)

    cb = np.zeros((128, BLOBW), f)

    def put(name, arr, packed=False):
        a, b_ = _OFF[name]
        arr = np.asarray(arr, f)
        v = _bf16_pack(arr) if packed else arr
        cb[0:v.shape[0], a:a + v.shape[1]] = v

    # c128: wredt bf16 [128, 16] | screl f32 [16, 2] | wspan_aug bf16 [17, 196]
    wsa = np.zeros((17, KK * G), f)
    ws3 = w_span.reshape(G, KK, CR)          # [g, kk, r]
    wsa[0:16] = ws3.transpose(2, 1, 0).reshape(CR, KK * G)
    wsa[16] = b_span.reshape(G, KK).T.reshape(KK * G)
    wrt = np.zeros((128, CR), f)
    wrt[0:64] = (w_reduce[:, perm] * sc_r[:, None]).T
    wrt[64:128] = wrt[0:64]
    c128 = np.zeros((128, 108), f)
    c128[:, 0:8] = _bf16_pack(wrt)
    c128[0:16, 8:10] = np.stack([sc_r, bi_r], axis=1)
    c128[0:17, 10:108] = _bf16_pack(wsa)
    put("c128", c128)

    wc = (w_conv[:, perm] * sc_c[:, None]).T
    wm = (w_map[:, perm] * sc_m[:, None]).T
    put("wce", np.concatenate([wc, wm], axis=0), packed=True)
    put("btail", (bi_c + sc_m * b_map + bi_m)[:, None])
    c1b = np.zeros((1, 448), f)
    c1b[:, 0:384] = _bf16_pack(np.tile(bi_i[perm], SEGI * NS)[None, :])
    c1b[:, 384:448] = _bf16_pack(np.ones((1, CO), f))
    put("c1b", c1b)
    put("tones", np.ones((1, PXW), f), packed=True)
    put("iden", np.eye(CO, dtype=f), packed=True)
    return cb, sc_i


def kernel(**inputs):
    x = np.asarray(inputs["x"], dtype=np.float32)
    assert x.shape == (B, C, H, W)

    # channel permutation c' = u*4 + g  (group index innermost)
    perm = np.array([(cp % G) * GC + cp // G for cp in range(C)], np.int64)

    if "cb" not in _CACHE:
        cb, sc_i = _prep_blob_consts(
            **{k: np.asarray(v) for k, v in inputs.items() if k != "x"},
            perm=perm)
        _CACHE["cb"] = cb
        _CACHE["sc_i"] = sc_i
    cb, sc_i = _CACHE["cb"], _CACHE["sc_i"]

    if "nc" not in _CACHE:
        _CACHE["nc"] = _build_bass()
    nc = _CACHE["nc"]

    xp = x[:, perm, :, :]                        # [B, c', H, W]
    xs_scaled = xp * sc_i[perm][None, :, None, None]

    rows_idx = (RBR * np.arange(NRB)[:, None]
                + np.arange(IROWS)[None, :])     # [rb, ii]

    in_maps = []
    for core in range(NCORES):
        b, half = core // 2, core % 2
        r0 = half * ROWS
        blob = cb.copy()
        # padded x*s_i for this core's row window
        xpad = np.zeros((C, ROWS + 2 * PAD, W + 2 * PAD), np.float32)
        glo, ghi = max(r0 - PAD, 0), min(r0 + ROWS + PAD, H)
        xpad[:, glo - (r0 - PAD):ghi - (r0 - PAD), PAD:PAD + W] = (
            xs_scaled[b, :, glo:ghi, :])
        for kj in range(KS):
            sub = xpad[:, :, kj:kj + W]          # [c, 54, 96]
            t4 = sub.reshape(C, ROWS + 2 * PAD, NS, SW)[:, rows_idx]
            # t4: [c, rb, ii, s, js] -> [(rb, js), (ii, s, c)]
            xs_kj = t4.transpose(1, 4, 2, 3, 0).reshape(128, IROWS * NS * C)
            a0, a1 = _OFF[f"xs{kj}"]
            blob[:, a0:a1] = _bf16_pack(xs_kj)
        # channel-major x (raw) for t: halves of 24 rows across partitions,
        # rows ordered (q, b', e, j) so each t-chunk rhs is contiguous
        xcm = np.empty((128, RBR * 2 * W), np.float32)
        for h in range(2):
            rows = xp[b, :, r0 + 24 * h:r0 + 24 * h + 24, :]
            xcm[64 * h:64 * h + 64] = (
                rows.reshape(C, 2, 6, 2, W).transpose(0, 2, 1, 3, 4)
                .reshape(C, 24 * W))
        a0, a1 = _OFF["xcm"]
        blob[:, a0:a1] = _bf16_pack(xcm)
        # channel-major x for the tail map branch (partitions 64-127)
        a0, a1 = _OFF["xcmt"]
        blob[64:128, a0:a1] = _bf16_pack(
            xp[b, :, r0:r0 + ROWS, :].reshape(C, PXW))
        in_maps.append({"blob": blob})

    res = run_bass_kernel_spmd(nc, in_maps, core_ids=list(range(NCORES)))

    out = np.empty((B, CO, H, W), np.float32)
    for core in range(NCORES):
        b, half = core // 2, core % 2
        o = np.asarray(res.results[core]["out"]).astype(np.float32)
        out[b, :, half * ROWS:(half + 1) * ROWS, :] = o.reshape(CO, ROWS, W)
    if KDEBUG:
        _CACHE["dbg"] = res.results
    return out


# revision 12
# speedup vs baseline: 1.0002x; 1.0002x over previous
"""Trainium2 Bass kernel for the BottleNeck-involution block — 128-partition
repack.

Sharding: pure data parallel over (batch=4) x (H halves) = 8 shards, one per
NeuronCore.  Each core computes a (1, 128, 48, 96) slice of the output.

Layout: partitions hold (rb, js): rb = 4 row-blocks of 12 rows, js = 32 cols
of a strip; 3 strips s cover the 96 cols.  All 128 partitions carry involution
work (vs 96 in the j-only packing), so every engine's per-element cost drops
by 4/3.  Free dim rasters:
  xs[kj]  [128, (ii 18, s 3, c' 64)]   x*s_i, 7 col-shifted copies (bf16)
  kern_s  [128, (i 12, s 3, kk*4+g)]   per-pixel involution kernels (bf16)
  acc     [128, (i 4, s 3, c' 64)]     PSUM accumulator, one of 3 row-segs
  t_sb    [17, (i 48, j 96)]           reduced features + ones row
  rhs     [128=(x1_cm 64 | x_cm 64), (i, j)]  tail 1x1-conv operand

Per-core pipeline:
  t       = relu(bn_r(w_reduce @ x))     PE + ACT  (channel-major, 6 chunks)
  kern    = [t;1]^T @ [wspan; bspan]     PE        (36 units, (rb,js)-packed)
  3 segs x 49 taps:
    pr    = kern(bcast) * xs[kj]         DVE (38) / Pool (11)  bf16 2x mode
    acc   = b_i + sum pr                 PE identity-matmul PSUM accum
    x1    = gelu(acc)                    ACT
  tail (per seg, interleaved into next seg's taps):
    x1_cm = transpose(x1)                PE + ACT copies
    out   = gelu(wce @ [x1;x] + btail)   PE + ACT, DMA out
"""

import os
import sys

sys.path.insert(0, "/opt/trn_rl_repo")
KDEBUG = int(os.environ.get("KDEBUG", "0"))

import numpy as np

import concourse.bass as bass
from concourse import bacc
import concourse.mybir as mybir
import concourse.tile as tile
from concourse.bass_utils import run_bass_kernel_spmd

F32 = mybir.dt.float32
BF16 = mybir.dt.bfloat16

EPS = 1e-5
KS = 7            # involution kernel size
KK = KS * KS      # 49 taps
GC = 16           # channels per involution group
G = 4             # groups
CR = 16           # reduced channels
B, C, H, W = 4, 64, 96, 96
CO = 128
NCORES = 8

ROWS = H // 2     # 48 output rows per core
PAD = 3
NRB = 4           # row-blocks per core
RBR = ROWS // NRB  # 12 rows per block
NS = 3            # col strips
SW = W // NS      # 32 cols per strip... (strip width must divide 128/NRB)
IROWS = RBR + 2 * PAD  # 18 input rows per block (with halo)
NSEG = 3
SEGI = RBR // NSEG     # 4 i_rel rows per segment
SEGW = SEGI * NS * C   # 768 acc cols per segment
PXW = ROWS * W         # 4608 pixels per core

# taps in kj-major order so products start as each shifted x copy lands
TAPS = [(ki, kj) for kj in range(KS) for ki in range(KS)]
NPOOL = 11
POOL_SLOTS = frozenset(round(3 + m * 45 / (NPOOL - 1)) for m in range(NPOOL))
# last segment runs one fewer Pool tap so Pool drains before DVE
POOL_SLOTS_LAST = frozenset(round(3 + m * 45 / 9) for m in range(10))
POOL_BY_SEG = (POOL_SLOTS, POOL_SLOTS, POOL_SLOTS_LAST)

# blob f32-slot layout: name -> (width, partitions)
_REGIONS = [
    ("xcm", RBR * W, 128),            # x channel-major bf16 [128, 24*96]
    ("c128", 108, 128),               # wredt bf16 | screl f32 | wspan bf16
    ("tones", PXW // 2, 1),           # ones bf16 [1, 4608] (t_sb row 16)
    ("c1b", 448, 1),                  # birow bf16 | ones1 bf16
    ("iden", CO // 2, 128),           # identity bf16 [128, 128]
] + [
    (f"xs{kj}", IROWS * NS * C // 2, 128) for kj in range(KS)
] + [
    ("xcmt", PXW // 2, 128),          # x channel-major bf16 at parts 64-127
    ("wce", CO // 2, 128),            # [wconv; wmap] stacked bf16 [128, 128]
    ("btail", 1, 128),                # tail gelu bias f32 [128, 1]
]
_OFF = {}
_o = 0
for _name, _w, _p in _REGIONS:
    _OFF[_name] = (_o, _o + _w)
    _o += _w
BLOBW = _o

_CACHE = {}


def _build_bass():
    nc = bacc.Bacc()

    blob_d = nc.dram_tensor("blob", [128, BLOBW], F32, kind="ExternalInput")
    out_d = nc.dram_tensor("out", [CO, PXW], BF16, kind="ExternalOutput")
    if KDEBUG:
        tdbg_d = nc.dram_tensor("tdbg", [17, PXW // 2], F32, kind="ExternalOutput")
        kdbg_d = nc.dram_tensor(
            "kdbg", [128, RBR * NS * KK * G // 2], F32, kind="ExternalOutput")
        xdbg_d = nc.dram_tensor(
            "xdbg", [128, RBR * NS * C // 2], F32, kind="ExternalOutput")

    def breg(name, p0=0, p1=None):
        a, b_ = _OFF[name]
        np_ = dict((n, pp) for n, _, pp in _REGIONS)[name] if p1 is None else p1
        return blob_d[p0:np_, a:b_]

    with tile.TileContext(nc) as tc:
        with (
            tc.tile_pool(name="work", bufs=1) as work,
            tc.tile_pool(name="prodD", bufs=10) as prodD,
            tc.tile_pool(name="prodP", bufs=6) as prodP,
            tc.tile_pool(name="outp", bufs=3) as outp,
        ):
            # ---- SBUF tiles + loads (DMA queue order = priority order) ----
            iden = work.tile([128, CO], BF16, name="iden")
            nc.sync.dma_start(iden[:].bitcast(F32), breg("iden"))
            x_cm = work.tile([128, RBR * 2 * W], BF16, name="x_cm")
            nc.sync.dma_start(x_cm[:].bitcast(F32), breg("xcm"))
            c128 = work.tile([128, 108], F32, name="c128")
            nc.sync.dma_start(c128[:], breg("c128"))
            wredt = c128[:, 0:8].bitcast(BF16)
            screl = c128[0:16, 8:10]
            wspan = c128[0:17, 10:108].bitcast(BF16)
            t_sb = work.tile([17, PXW], BF16, name="t_sb")
            nc.sync.dma_start(t_sb[16:17, :].bitcast(F32), breg("tones"))

            xs = []
            for kj in range(KS):
                t_ = work.tile([128, IROWS * NS * C], BF16, name=f"xs{kj}")
                xs.append(t_)

            nc.sync.dma_start(xs[0][:].bitcast(F32), breg("xs0"))
            c1b = work.tile([1, 448], F32, name="c1b")
            nc.sync.dma_start(c1b[:], breg("c1b"))
            birow = c1b[:, 0:384].bitcast(BF16)
            ones1 = c1b[:, 384:448].bitcast(BF16)
            for kj in range(1, KS):
                nc.sync.dma_start(xs[kj][:].bitcast(F32), breg(f"xs{kj}"))

            rhs = work.tile([128, PXW], BF16, name="rhs")
            a0, a1 = _OFF["xcmt"]
            nc.sync.dma_start(
                rhs[64:128, :].bitcast(F32), blob_d[64:128, a0:a1])
            wce = work.tile([128, CO], BF16, name="wce")
            nc.sync.dma_start(wce[:].bitcast(F32), breg("wce"))
            btail = work.tile([128, 1], F32, name="btail")
            nc.sync.dma_start(btail[:], breg("btail"))

            kern_s = work.tile([128, RBR * NS * KK * G], BF16, name="kern_s")
            x1_pm = work.tile([128, RBR * NS * C], BF16, name="x1_pm")
            scr = work.tile([16, 1], F32, name="scr")

            # ---- PSUM pools (LIFO): acc > kps > tps ----
            acc_cm = tc.tile_pool(name="accp", bufs=2, space="PSUM")
            accp = acc_cm.__enter__()
            kps_cm = tc.tile_pool(name="kpsp", bufs=2, space="PSUM")
            kpsp = kps_cm.__enter__()
            tps_cm = tc.tile_pool(name="tpsp", bufs=2, space="PSUM")
            tpsp = tps_cm.__enter__()

            # dummy gelu: force the gelu_and_others act table load at t=0
            nc.scalar.activation(
                scr[:, 0:1], screl[:, 0:1],
                mybir.ActivationFunctionType.Gelu,
            )

            # PE warmup: ramp the tensor engine to full p-state during the
            # x_cm DMA wait (iden lands first; ~20 x 128-cycle matmuls)
            wrm = tpsp.tile([16, 384], F32, name="tps", tag="tps")
            for _ in range(17):
                nc.tensor.matmul(
                    wrm[0:16, 0:128], iden[:, 0:16], iden[:],
                    start=True, stop=True,
                )

            # t_sb free raster: (i_rel 12, s 3, rb 4, js 32) — kern-unit slices
            # are contiguous 128-pixel runs (matmul APs allow 1 free dim)
            t_pk = t_sb[:].rearrange("p (i s r j) -> p r i s j",
                                     i=RBR, s=NS, r=NRB)

            def t_chunk(q, dve_relu=False):
                """t rows {12b' + 2q + e} of both halves (x_cm is host-ordered
                (q, b', e, j) per half so the rhs slice is contiguous).
                bn_r scale is folded into wredt on the host; relu applies the
                bias, on ACT or (dve_relu, for halves h=1) DVE."""
                for h in range(2):
                    tp = tpsp.tile([16, 384], F32, name="tps", tag="tps")
                    nc.tensor.matmul(
                        tp[:],
                        wredt[64 * h:64 * h + 64, :],
                        x_cm[64 * h:64 * h + 64, 384 * q:384 * (q + 1)],
                        start=True, stop=True,
                    )
                    dst = t_pk[0:16, 2 * h:2 * h + 2, 2 * q:2 * q + 2, :, :]
                    if dve_relu and h == 1:
                        nc.vector.tensor_scalar(
                            out=dst, in0=tp[:],
                            scalar1=screl[:, 1:2], scalar2=0.0,
                            op0=mybir.AluOpType.add,
                            op1=mybir.AluOpType.max,
                        )
                    else:
                        nc.scalar.activation(
                            dst, tp[:],
                            mybir.ActivationFunctionType.Relu,
                            bias=screl[:, 1:2],
                            scale=1.0,
                        )

            def kern_pair(p, dve_copy=False):
                """produce kern units 2p, 2p+1 (196 cols each)."""
                kp = kpsp.tile([128, 392], F32, name="kps", tag="kps")
                for s_ in range(2):
                    u = 2 * p + s_
                    nc.tensor.matmul(
                        kp[:, s_ * KK * G:(s_ + 1) * KK * G],
                        t_sb[0:17, 128 * u:128 * (u + 1)],
                        wspan,
                        start=True, stop=True,
                    )
                dst = kern_s[:, 2 * p * KK * G:(2 * p + 2) * KK * G]
                if dve_copy:
                    nc.vector.tensor_copy(dst, kp[:])
                else:
                    nc.scalar.activation(
                        dst, kp[:], mybir.ActivationFunctionType.Identity)

            # ---- lead-in: t chunks 0-1, kern units 0-11 ----
            t_chunk(0)
            kern_pair(0, dve_copy=True)
            kern_pair(1, dve_copy=True)
            t_chunk(1, dve_relu=True)
            kern_pair(2, dve_copy=True)

            # tail pools opened lazily after kps/tps close
            tail_pools = {}

            def tail_a(seg, uu, dve_copy, eng=None):
                """transpose + copy for unit uu of segment seg."""
                tp2p = tail_pools["tp2p"]
                ig = 4 * seg + uu // NS
                st = uu % NS
                ucol = (ig * NS + st) * C
                tp2 = tp2p.tile([64, 128], BF16, name="tp2", tag="tp2")
                nc.tensor.transpose(
                    tp2[:], x1_pm[:, ucol:ucol + C], iden[:])
                dst = rhs[0:64, :].rearrange(
                    "p (b r j) -> p b r j", b=NRB, r=RBR
                )[:, :, ig, SW * st:SW * st + SW]
                if eng is not None:
                    eng.tensor_copy(dst, tp2[:])
                elif dve_copy:
                    nc.vector.tensor_copy(dst, tp2[:])
                else:
                    nc.scalar.activation(
                        dst, tp2[:], mybir.ActivationFunctionType.Identity)

            def tail_b(seg, rb, dma_eng=None):
                """1x1 convs + gelu + out DMA for rows 12rb+4seg..+4."""
                tlp = tail_pools["tlp"]
                px0 = (RBR * rb + SEGI * seg) * W
                pxw = SEGI * W
                tl = tlp.tile([128, pxw], F32, name="tailps", tag="tailps")
                nc.tensor.matmul(
                    tl[:], wce[:], rhs[:, px0:px0 + pxw],
                    start=True, stop=True,
                )
                ost = outp.tile([128, pxw], BF16, name="ost", tag="ost")
                nc.scalar.activation(
                    ost[:], tl[:],
                    mybir.ActivationFunctionType.Gelu,
                    bias=btail[:],
                    scale=1.0,
                )
                (dma_eng or nc.sync).dma_start(out_d[:, px0:px0 + pxw], ost[:])

            def product(seg, slot, pr, i0, i1):
                """tap product for i_loc range [i0, i1) into pr's slice."""
                ki, kj = TAPS[slot]
                kk = ki * KS + kj
                n = i1 - i0
                pr4 = pr[:].rearrange(
                    "p (i s u g) -> p i s u g", i=SEGI, s=NS, u=GC
                )[:, i0:i1]
                b0 = SEGI * seg + ki + i0
                in0 = xs[kj][:].rearrange(
                    "p (i s c) -> p i s c", i=IROWS, s=NS
                )[:, b0:b0 + n, :, :].rearrange("p i s (u g) -> p i s u g", g=G)
                in1 = kern_s[:].rearrange(
                    "p (i s k) -> p i s k", i=RBR, s=NS
                )[:, SEGI * seg + i0:SEGI * seg + i1, :, kk * G:(kk + 1) * G]
                in1b = in1.unsqueeze(3).to_broadcast([128, n, NS, GC, G])
                eng = nc.gpsimd if slot in POOL_BY_SEG[seg] else nc.vector
                eng.tensor_tensor(out=pr4, in0=in0, in1=in1b,
                                  op=mybir.AluOpType.mult)

            # ---- segments ----
            for seg in range(NSEG):
                acc = accp.tile([128, SEGW], F32, name="acc", tag="acc")
                for c0, c1 in ((0, 512), (512, SEGW)):
                    nc.tensor.matmul(
                        acc[:, c0:c1], ones1, birow[:, c0:c1],
                        start=True, stop=False,
                    )
                pend = []
                lag = [0]

                def flush(limit, acc=acc, pend=pend):
                    while len(pend) > limit:
                        pr_, last_ = pend.pop(0)
                        for c0, c1 in ((0, 512), (512, SEGW)):
                            nc.tensor.matmul(
                                acc[:, c0:c1], iden[:], pr_[:, c0:c1],
                                start=False, stop=last_,
                            )

                first = 0
                if seg == 0:
                    # startup: all kj0 taps as half-products — half A needs
                    # only kern pairs 0-2, so the vec engines start as soon
                    # as xs0 lands (seg0 is xs-DMA-paced; the extra per-op
                    # init hides in the DMA wait). Pairs 3-5 copy on ACT in
                    # parallel, then half B.
                    first = KS
                    prs = []
                    for slot in range(KS):
                        pp = prodP if slot in POOL_SLOTS else prodD
                        pr = pp.tile([128, SEGW], BF16, name="prod",
                                     tag="prod")
                        prs.append(pr)
                        product(0, slot, pr, 0, SEGI // 2)
                    for p in range(3, 6):
                        kern_pair(p, dve_copy=False)
                    t_chunk(2)
                    t_chunk(3)
                    for slot in range(KS):
                        product(0, slot, prs[slot], SEGI // 2, SEGI)
                        pend.append((prs[slot], slot == 98))
                        if slot in POOL_SLOTS:
                            lag[0] = 2
                        flush(lag[0])
                        if lag[0] > 0 and slot not in POOL_SLOTS:
                            lag[0] -= 1

                for slot in range(first, KK):
                    pool_tap = slot in POOL_BY_SEG[seg]
                    pp = prodP if pool_tap else prodD
                    pr = pp.tile([128, SEGW], BF16, name="prod", tag="prod")
                    product(seg, slot, pr, 0, SEGI)
                    pend.append((pr, slot == KK - 1))
                    if pool_tap:
                        lag[0] = 2
                    flush(lag[0])
                    if lag[0] > 0 and not pool_tap:
                        lag[0] -= 1

                    # interleaved extras
                    if seg == 0:
                        if 7 <= slot < 9:
                            t_chunk(slot - 3)
                        if slot >= 7 and (slot - 7) % 3 == 0 \
                                and (slot - 7) // 3 < 12:
                            kern_pair(6 + (slot - 7) // 3, dve_copy=False)
                    elif "tp2p" in tail_pools:
                        ps = seg - 1
                        if slot % 3 == 1 and slot // 3 < 12:
                            tail_a(ps, slot // 3, dve_copy=False)
                        if slot >= 37 and (slot - 37) % 3 == 0 \
                                and (slot - 37) // 3 < 4:
                            tail_b(ps, (slot - 37) // 3)
                flush(0)
                for g0, g1 in ((0, 192), (192, 384), (384, SEGW)):
                    nc.scalar.activation(
                        x1_pm[:, SEGW * seg + g0:SEGW * seg + g1],
                        acc[:, g0:g1],
                        mybir.ActivationFunctionType.Gelu,
                    )
                if seg == 0:
                    # kern + t complete: close tps/kps, open tail pools
                    tps_cm.__exit__(None, None, None)
                    kps_cm.__exit__(None, None, None)
                    tl_cm = tc.tile_pool(name="tlp", bufs=1, space="PSUM")
                    tail_pools["tlp"] = tl_cm.__enter__()
                    tail_pools["tl_cm"] = tl_cm
                    tp2_cm = tc.tile_pool(name="tp2p", bufs=3, space="PSUM")
                    tail_pools["tp2p"] = tp2_cm.__enter__()
                    tail_pools["tp2_cm"] = tp2_cm

            # trailing tail (segment 2): every out chunk needs all 12
            # units; copies split 3-way (DVE/ACT/Pool all idle by now).
            for uu in range(RBR):
                tail_a(NSEG - 1, uu, dve_copy=(uu % 6 != 0))
            # out chunks in 2 groups of 2; each group's matmuls land in one
            # 2-bank acc-pool tile (dsts at 0 and 512 to stay bank-aligned),
            # then a single strided-source gelu and a single batched DMA.
            ostf = outp.tile([128, NRB * SEGI * W], BF16, name="ostf",
                             tag="ostf")
            pxw = SEGI * W
            for grp in range(2):
                tl = accp.tile([128, 1024], F32, name="acc", tag="acc")
                for k in range(2):
                    rb = 2 * grp + k
                    px0 = (RBR * rb + SEGI * (NSEG - 1)) * W
                    nc.tensor.matmul(
                        tl[:, 512 * k:512 * k + pxw], wce[:],
                        rhs[:, px0:px0 + pxw],
                        start=True, stop=True,
                    )
                gsrc = tl[:].rearrange("p (k x) -> p k x", k=2)[:, :, 0:pxw]
                nc.scalar.activation(
                    ostf[:, grp * 2 * pxw:(grp + 1) * 2 * pxw], gsrc,
                    mybir.ActivationFunctionType.Gelu,
                    bias=btail[:],
                    scale=1.0,
                )
                odst = out_d[:].rearrange(
                    "p (rb x) -> p rb x", rb=NRB
                )[:, 2 * grp:2 * grp + 2, SEGI * (NSEG - 1) * W:]
                nc.sync.dma_start(
                    odst, ostf[:, grp * 2 * pxw:(grp + 1) * 2 * pxw])

            if KDEBUG:
                nc.sync.dma_start(tdbg_d[:, :], t_sb[:].bitcast(F32))
                nc.sync.dma_start(kdbg_d[:, :], kern_s[:].bitcast(F32))
                nc.sync.dma_start(xdbg_d[:, :], x1_pm[:].bitcast(F32))

            tail_pools["tp2_cm"].__exit__(None, None, None)
            tail_pools["tl_cm"].__exit__(None, None, None)
            acc_cm.__exit__(None, None, None)

    if not nc.is_finalized():
        nc.finalize()
    return nc


def _bf16_pack(arr):
    """bf16-cast a [P, N] array and pack into [P, N/2] f32 slots."""
    import ml_dtypes

    a = np.ascontiguousarray(np.asarray(arr, np.float32)).astype(ml_dtypes.bfloat16)
    return a.view(np.float32)


def _prep_blob_consts(w_reduce, g_r, b_r, m_r, v_r, w_span, b_span,
                      g_i, b_i, m_i, v_i, w_conv, g_c, b_c, m_c, v_c,
                      w_map, b_map, g_m, b_m, m_m, v_m, perm):
    f = np.float32

    def bn_fold(g, b, m, v):
        s = g / np.sqrt(v + EPS)
        return s.astype(f), (b - m * s).astype(f)

    sc_r, bi_r = bn_fold(g_r, b_r, m_r, v_r)
    sc_i, bi_i = bn_fold(g_i, b_i, m_i, v_i)
    sc_c, bi_c = bn_fold(g_c, b_c, m_c, v_c)
    sc_m, bi_m = bn_fold(g_m, b_m, m_m, v_m)

    cb = np.zeros((128, BLOBW), f)

    def put(name, arr, packed=False):
        a, b_ = _OFF[name]
        arr = np.asarray(arr, f)
        v = _bf16_pack(arr) if packed else arr
        cb[0:v.shape[0], a:a + v.shape[1]] = v

    # c128: wredt bf16 [128, 16] | screl f32 [16, 2] | wspan_aug bf16 [17, 196]
    wsa = np.zeros((17, KK * G), f)
    ws3 = w_span.reshape(G, KK, CR)          # [g, kk, r]
    wsa[0:16] = ws3.transpose(2, 1, 0).reshape(CR, KK * G)
    wsa[16] = b_span.reshape(G, KK).T.reshape(KK * G)
    wrt = np.zeros((128, CR), f)
    wrt[0:64] = (w_reduce[:, perm] * sc_r[:, None]).T
    wrt[64:128] = wrt[0:64]
    c128 = np.zeros((128, 108), f)
    c128[:, 0:8] = _bf16_pack(wrt)
    c128[0:16, 8:10] = np.stack([sc_r, bi_r], axis=1)
    c128[0:17, 10:108] = _bf16_pack(wsa)
    put("c128", c128)

    wc = (w_conv[:, perm] * sc_c[:, None]).T
    wm = (w_map[:, perm] * sc_m[:, None]).T
    put("wce", np.concatenate([wc, wm], axis=0), packed=True)
    put("btail", (bi_c + sc_m * b_map + bi_m)[:, None])
    c1b = np.zeros((1, 448), f)
    c1b[:, 0:384] = _bf16_pack(np.tile(bi_i[perm], SEGI * NS)[None, :])
    c1b[:, 384:448] = _bf16_pack(np.ones((1, CO), f))
    put("c1b", c1b)
    put("tones", np.ones((1, PXW), f), packed=True)
    put("iden", np.eye(CO, dtype=f), packed=True)
    return cb, sc_i


def kernel(**inputs):
    x = np.asarray(inputs["x"], dtype=np.float32)
    assert x.shape == (B, C, H, W)

    # channel permutation c' = u*4 + g  (group index innermost)
    perm = np.array([(cp % G) * GC + cp // G for cp in range(C)], np.int64)

    if "cb" not in _CACHE:
        cb, sc_i = _prep_blob_consts(
            **{k: np.asarray(v) for k, v in inputs.items() if k != "x"},
            perm=perm)
        _CACHE["cb"] = cb
        _CACHE["sc_i"] = sc_i
    cb, sc_i = _CACHE["cb"], _CACHE["sc_i"]

    if "nc" not in _CACHE:
        _CACHE["nc"] = _build_bass()
    nc = _CACHE["nc"]

    xp = x[:, perm, :, :]                        # [B, c', H, W]
    xs_scaled = xp * sc_i[perm][None, :, None, None]

    rows_idx = (RBR * np.arange(NRB)[:, None]
                + np.arange(IROWS)[None, :])     # [rb, ii]

    in_maps = []
    for core in range(NCORES):
        b, half = core // 2, core % 2
        r0 = half * ROWS
        blob = cb.copy()
        # padded x*s_i for this core's row window
        xpad = np.zeros((C, ROWS + 2 * PAD, W + 2 * PAD), np.float32)
        glo, ghi = max(r0 - PAD, 0), min(r0 + ROWS + PAD, H)
        xpad[:, glo - (r0 - PAD):ghi - (r0 - PAD), PAD:PAD + W] = (
            xs_scaled[b, :, glo:ghi, :])
        for kj in range(KS):
            sub = xpad[:, :, kj:kj + W]          # [c, 54, 96]
            t4 = sub.reshape(C, ROWS + 2 * PAD, NS, SW)[:, rows_idx]
            # t4: [c, rb, ii, s, js] -> [(rb, js), (ii, s, c)]
            xs_kj = t4.transpose(1, 4, 2, 3, 0).reshape(128, IROWS * NS * C)
            a0, a1 = _OFF[f"xs{kj}"]
            blob[:, a0:a1] = _bf16_pack(xs_kj)
        # channel-major x (raw) for t: halves of 24 rows across partitions,
        # rows ordered (q, b', e, j) so each t-chunk rhs is contiguous
        xcm = np.empty((128, RBR * 2 * W), np.float32)
        for h in range(2):
            rows = xp[b, :, r0 + 24 * h:r0 + 24 * h + 24, :]
            xcm[64 * h:64 * h + 64] = (
                rows.reshape(C, 2, 6, 2, W).transpose(0, 2, 1, 3, 4)
                .reshape(C, 24 * W))
        a0, a1 = _OFF["xcm"]
        blob[:, a0:a1] = _bf16_pack(xcm)
        # channel-major x for the tail map branch (partitions 64-127)
        a0, a1 = _OFF["xcmt"]
        blob[64:128, a0:a1] = _bf16_pack(
            xp[b, :, r0:r0 + ROWS, :].reshape(C, PXW))
        in_maps.append({"blob": blob})

    res = run_bass_kernel_spmd(nc, in_maps, core_ids=list(range(NCORES)))

    out = np.empty((B, CO, H, W), np.float32)
    for core in range(NCORES):
        b, half = core // 2, core % 2
        o = np.asarray(res.results[core]["out"]).astype(np.float32)
        out[b, :, half * ROWS:(half + 1) * ROWS, :] = o.reshape(CO, ROWS, W)
    if KDEBUG:
        _CACHE["dbg"] = res.results
    return out
